# revision 1
# baseline (speedup 1.0000x reference)
"""Trainium2 Bass kernel for CausalSelfAttentionARMA (eval forward).

Sharding: 8 cores = 2 batches x 4 head-groups (4 heads each, d=64). Each core
computes its (batch, head-group) shard end-to-end and returns a transposed
partial output [C, T] in bf16; the host sums partials per batch and adds
2*b_proj.

Device-side structure (v3):
  - all DRAM traffic bf16; x rows are host-rolled so the core's own 256
    channel rows sit at xT[0:256] (W rows rolled identically).
  - heads in PAIRS: q/k/k2 tiles hold head A on partitions 0-63, head B on
    64-127; the two K=64 QK matmuls run CONCURRENTLY via tile_position row
    tiling. AV runs full-array per head with a ones-column producing the
    softmax denominators in psum row 64.
  - per g, unnormalized y is copied to SBUF and the denominator row to a
    per-pass [8, 512] tile. At pass end: 4 PE transposes -> one [128, 32]
    DVE reciprocal -> 4 PE transposes back -> selector-matmul broadcast
    into a [128, 512] psum tile (two heads as concurrent col tiles) ->
    in-place normalize muls. No single-lane reciprocals, no gpsimd.
  - e = vshift - y computed in the [d, t] domain from resident x rows,
    PE-transposed pair-packed into per-head eaug tiles, interleaved into
    the pass-2 g-loop.
  - exp windows causal-trimmed via strided 2-up APs.
"""
import numpy as np
import ml_dtypes

import concourse.bass as bass
import concourse.bacc as bacc
import concourse.tile as tile
from concourse import mybir
from concourse.bass_utils import run_bass_kernel_spmd

F32 = mybir.dt.float32
BF16 = mybir.dt.bfloat16

B, T, C = 2, 2048, 1024
H, D = 16, 64
HPC = 4                # heads per core
DG = HPC * D           # 256
NT = T // 128          # 16 t-tiles
NG = 4                 # q groups of 512
NP = 2                 # head pairs per core

_BUILT = None


def _build():
    nc = bacc.Bacc("TRN2", target_bir_lowering=False, debug=False, num_devices=8)

    xT = nc.declare_dram_parameter("xT", [C, T], BF16, isOutput=False)
    vaug = nc.declare_dram_parameter("vaug", [HPC * 128, NT * 128], BF16,
                                     isOutput=False)
    wq = nc.declare_dram_parameter("wq", [128, 8 * DG], BF16, isOutput=False)
    wk = nc.declare_dram_parameter("wk", [128, 8 * DG], BF16, isOutput=False)
    wk2 = nc.declare_dram_parameter("wk2", [128, 8 * DG], BF16, isOutput=False)
    wp = nc.declare_dram_parameter("wp", [128, 2 * C], BF16, isOutput=False)
    bq = nc.declare_dram_parameter("bq", [128, NP], F32, isOutput=False)
    bk = nc.declare_dram_parameter("bk", [128, NP], F32, isOutput=False)
    bk2 = nc.declare_dram_parameter("bk2", [128, NP], F32, isOutput=False)
    maskp = nc.declare_dram_parameter("mask2", [128, 256], BF16, isOutput=False)
    identp = nc.declare_dram_parameter("identb", [128, 128], BF16,
                                       isOutput=False)
    onesp = nc.declare_dram_parameter("ones4", [128, 4, 1], BF16,
                                      isOutput=False)
    selp = nc.declare_dram_parameter("sel32", [32, 2048], BF16, isOutput=False)
    outT = nc.declare_dram_parameter("outT", [C, T], BF16, isOutput=True)

    with tile.TileContext(nc) as tc:
        import contextlib
        with contextlib.ExitStack() as ctx:
            const = ctx.enter_context(tc.tile_pool(name="const", bufs=1))
            persist = ctx.enter_context(tc.tile_pool(name="persist", bufs=1))
            small = ctx.enter_context(tc.tile_pool(name="small", bufs=3))

            # small consts early on the sync queue (cheap)
            bias_sb = {}
            for nm, par in (("bq", bq), ("bk", bk), ("bk2", bk2)):
                t = const.tile([128, NP], F32, tag=nm, name=nm)
                nc.sync.dma_start(t[:], par[:])
                bias_sb[nm] = t
            sel_sb = const.tile([32, 2048], BF16, tag="sel")
            nc.sync.dma_start(sel_sb[:], selp[:])
            mask_sb = const.tile([128, 256], BF16, tag="mask")
            ident_sb = const.tile([128, 128], BF16, tag="ident")
            wp_sb = const.tile([128, 2 * C], BF16, tag="wp")

            vaug_sb = [persist.tile([128, NT * 128], BF16, tag=f"vaug{h}",
                                    name=f"vaug{h}") for h in range(HPC)]
            xpair = [persist.tile([128, T + 8], BF16, tag=f"xpair{p}",
                                  name=f"xpair{p}") for p in range(NP)]
            qpair = [persist.tile([128, T + 8], BF16, tag=f"q{p}", name=f"q{p}")
                     for p in range(NP)]
            kpair = [persist.tile([128, T], BF16, tag=f"k{p}", name=f"k{p}")
                     for p in range(NP)]
            k2pair = [persist.tile([128, T], BF16, tag=f"k2{p}", name=f"k2{p}")
                      for p in range(NP)]
            zT = [persist.tile([128, T], BF16, tag=f"zT{p}", name=f"zT{p}")
                  for p in range(NP)]
            y2un = [persist.tile([128, T], BF16, tag=f"y2un{p}",
                                 name=f"y2un{p}") for p in range(NP)]
            eaug_sb = [persist.tile([128, NT * 128], BF16, tag=f"eaug{h}",
                                    name=f"eaug{h}") for h in range(HPC)]

            # ---- shared SBUF pools for attention ----
            expp = ctx.enter_context(tc.tile_pool(name="expp", bufs=6))
            etmp = ctx.enter_context(tc.tile_pool(name="etmp", bufs=2))
            dpool = ctx.enter_context(tc.tile_pool(name="dpool", bufs=2))
            xs = ctx.enter_context(tc.tile_pool(name="xs", bufs=12))

            def emit_j(ps_pool, pyt, p, keysT, stats, qoff, g, j, jlast):
                col0 = max(0, (j - 4 * g) * 128)
                ps = ps_pool.tile([128, 1024], F32, tag="ps", name="ps")
                for hh in range(2):
                    nc.tensor.matmul(
                        ps[:, hh * 512 + col0:(hh + 1) * 512],
                        keysT[hh * 64:(hh + 1) * 64, j * 128:(j + 1) * 128],
                        qpair[p][hh * 64:(hh + 1) * 64,
                                 qoff + g * 512 + col0:qoff + (g + 1) * 512],
                        start=True, stop=True, tile_position=(hh * 64, 0))
                texp = expp.tile([128, 1024], BF16, tag="exp", name="texp")
                if col0 == 0:
                    nc.scalar.activation(
                        texp[:], ps[:],
                        mybir.ActivationFunctionType.Exp, scale=0.125)
                else:
                    pv = ps[:].rearrange("q (h w) -> q h w", w=512)
                    tv = texp[:].rearrange("q (h w) -> q h w", w=512)
                    nc.scalar.activation(
                        tv[:, :, col0:512], pv[:, :, col0:512],
                        mybir.ActivationFunctionType.Exp, scale=0.125)
                if j >= 4 * g:
                    tv = texp[:].rearrange("q (h w) -> q h w", w=512)
                    mv = mask_sb[:].rearrange("q (h w) -> q h w", w=128)
                    nc.vector.tensor_mul(
                        tv[:, :, col0:col0 + 128],
                        tv[:, :, col0:col0 + 128], mv[:])
                for hh in range(2):
                    nc.tensor.matmul(
                        pyt[hh][:, col0:512],
                        stats[hh][:, j * 128:(j + 1) * 128],
                        texp[:, hh * 512 + col0:(hh + 1) * 512],
                        start=(j == 0), stop=(j == jlast),
                        skip_group_check=True)

            def emit_end(dten, ydst, pyt, g):
                for hh in range(2):
                    nc.vector.tensor_copy(
                        dten[32 * g:32 * g + 1, hh * 512:(hh + 1) * 512],
                        pyt[hh][64:65, :])
                for hh in range(2):
                    nc.vector.tensor_copy(
                        ydst[hh * 64:hh * 64 + 64, g * 512:(g + 1) * 512],
                        pyt[hh][0:64, :])

            def group_units(ps_pool, py_pool, p, keysT, stats, qoff, ydst,
                            dten, g):
                """Emission units (closures) for attention group g."""
                state = {}
                jlast = 4 * g + 3
                units = [lambda: None, lambda: None]

                def first(j=0):
                    state["pyt"] = [
                        py_pool.tile([128, 512], F32, tag="py", name="py")
                        for _ in range(2)]
                    emit_j(ps_pool, state["pyt"], p, keysT, stats, qoff,
                           g, 0, jlast)
                units.append(first)
                for j in range(1, jlast + 1):
                    units.append(lambda j=j: emit_j(
                        ps_pool, state["pyt"], p, keysT, stats, qoff,
                        g, j, jlast))
                units.append(lambda: emit_end(dten, ydst, state["pyt"], g))
                return units

            # ---- phase A: projections woven with pair-0 pass-1 g0..g3 ----
            dten_p0 = dpool.tile([128, 1024], BF16, tag="dten", name="dten")
            from collections import deque
            wq_units = deque()

            def weave():
                if wq_units:
                    wq_units.popleft()()

            with tc.tile_pool(name="pproj", bufs=1, space="PSUM") as pproj, \
                 tc.tile_pool(name="psA", bufs=2, space="PSUM") as psA, \
                 tc.tile_pool(name="pyA", bufs=2, space="PSUM") as pyA:
                projs = []
                for nm, par in (("wq", wq), ("wk", wk), ("wk2", wk2)):
                    t = const.tile([128, 8 * DG], BF16, tag=nm, name=nm)
                    projs.append((t, par))
                wq_sb, wk_sb, wk2_sb = [t for t, _ in projs]

                def gp_extras(n):
                    if n == 0:
                        for h in (0, 1):
                            nc.gpsimd.dma_start(
                                vaug_sb[h][:], vaug[h * 128:(h + 1) * 128, :])
                    elif n == 1:
                        for h in (2, 3):
                            nc.gpsimd.dma_start(
                                vaug_sb[h][:], vaug[h * 128:(h + 1) * 128, :])
                        for p in range(NP):
                            nc.gpsimd.memset(xpair[p][:, T:T + 8], 0.0)
                            nc.gpsimd.dma_start(
                                xpair[p][:, 0:T], xT[p * 128:(p + 1) * 128, :])
                    elif n == 2:
                        for h in range(HPC):
                            nc.gpsimd.memset(eaug_sb[h][:], 0.0)

                plan = [(wq_sb, qpair, "bq"), (wk_sb, kpair, "bk")]
                for n in range(4):
                    xps = []
                    for pp in range(NP):
                        accs = {}
                        for c in range(8):
                            if pp == 0:
                                if n == 0:
                                    for t, par in projs:
                                        nc.sync.dma_start(
                                            t[:, c * DG:(c + 1) * DG],
                                            par[:, c * DG:(c + 1) * DG])
                                xp = xs.tile([128, 512], BF16, tag="xp",
                                             name="xp")
                                eng = nc.sync if (n * 8 + c) % 2 == 0 \
                                    else nc.gpsimd
                                eng.dma_start(
                                    xp[:], xT[c * 128:(c + 1) * 128,
                                              n * 512:(n + 1) * 512])
                                xps.append(xp)
                            for pi, (w_sb, dsts, bnm) in enumerate(plan):
                                if c == 0:
                                    accs[pi] = pproj.tile(
                                        [128, 512], F32, tag=f"acc{pi}",
                                        name=f"acc{pi}")
                                nc.tensor.matmul(
                                    accs[pi][:],
                                    w_sb[:, c * DG + pp * 128:
                                         c * DG + pp * 128 + 128],
                                    xps[c][:],
                                    start=(c == 0), stop=(c == 7))
                            weave()
                        for pi, (w_sb, dsts, bnm) in enumerate(plan):
                            nc.vector.tensor_scalar_add(
                                dsts[pp][:, n * 512:(n + 1) * 512],
                                accs[pi][:], bias_sb[bnm][:, pp:pp + 1])
                        weave()
                    if n == 0:
                        nc.sync.dma_start(mask_sb[:], maskp[:])
                        nc.sync.dma_start(ident_sb[:], identp[:])
                    gp_extras(n)
                    wq_units.extend(group_units(
                        psA, pyA, 0, kpair[0], (vaug_sb[0], vaug_sb[1]),
                        0, zT[0], dten_p0, n))
                while wq_units:
                    wq_units.popleft()()

            # pad col for shifted q (col T = col T-1; value discarded)
            for p in range(NP):
                nc.vector.tensor_copy(qpair[p][:, T:T + 1],
                                      qpair[p][:, T - 1:T])

            # ---- phase B pools ----
            actx = contextlib.ExitStack()
            ps_pool = actx.enter_context(
                tc.tile_pool(name="ps", bufs=2, space="PSUM"))
            py_pool = actx.enter_context(
                tc.tile_pool(name="py", bufs=3, space="PSUM"))
            aux_pool = actx.enter_context(
                tc.tile_pool(name="aux", bufs=1, space="PSUM"))

            def attn_pass(p, keysT, stats, qoff, ydst, ework=None,
                          between_g=None):
                """Phase-B causal attention for pair p (plain g loop);
                ework(g) thunks are spread across g-1's j-iterations;
                between_g[g]() is emitted after group g completes."""
                dten = dpool.tile([128, 1024], BF16, tag="dten", name="dten")
                for g in range(NG):
                    if ework is not None and g == 0:
                        for th in ework(0):
                            th()
                    pending = list(ework(g + 1)) if (
                        ework is not None and g + 1 < NG) else []
                    units = group_units(ps_pool, py_pool, p, keysT, stats,
                                        qoff, ydst, dten, g)
                    for k, u in enumerate(units):
                        u()
                        if pending and k < len(units) - 1:
                            pending.pop(0)()
                    while pending:
                        pending.pop(0)()
                    if between_g is not None:
                        between_g[g]()
                return dten

            def k2_slice(pp, n):
                """Project k2 for pair pp, t-slice n (phase B, one aux
                bank as the accumulator; x re-streamed from DRAM)."""
                def th():
                    acc = aux_pool.tile([128, 512], F32, tag="aux",
                                        name="k2acc")
                    for c in range(8):
                        xp = xs.tile([128, 512], BF16, tag="xp", name="xp")
                        eng = nc.sync if c % 2 == 0 else nc.gpsimd
                        eng.dma_start(
                            xp[:], xT[c * 128:(c + 1) * 128,
                                      n * 512:(n + 1) * 512])
                        nc.tensor.matmul(
                            acc[:],
                            wk2_sb[:, c * DG + pp * 128:
                                   c * DG + pp * 128 + 128],
                            xp[:], start=(c == 0), stop=(c == 7))
                    nc.vector.tensor_scalar_add(
                        k2pair[pp][:, n * 512:(n + 1) * 512],
                        acc[:], bias_sb["bk2"][:, pp:pp + 1])
                return th

            def normalize(p, dten, apply_fn):
                """Pass-end: reciprocal the 8 denominator rows and hand a
                [128, 512] psum tile (rows 0-63 head A recip, 64-127 head B)
                to apply_fn(g, rbc) for each g."""
                fwdps = aux_pool.tile([128, 1024], BF16, tag="aux",
                                      name="fwdps")
                for hh in range(2):
                    for b in range(4):
                        c0 = hh * 512 + b * 128
                        nc.tensor.transpose(
                            fwdps[:, c0:c0 + 128],
                            dten[:, c0:c0 + 128],
                            ident_sb[:])
                # useful fwdps cols: i*32 for i = hh*16 + b*4 + g
                fv = fwdps[:].rearrange("q (i c) -> q i c", c=32)[:, :, 0:1]
                rrec = small.tile([128, 32], BF16, tag="rrec", name="rrec")
                rv = rrec[:].rearrange("q (i c) -> q i c", c=1)
                with nc.allow_low_precision(reason="bf16 softmax denom recip"):
                    nc.vector.reciprocal(rv, fv)
                backps = aux_pool.tile([32, 128], BF16, tag="aux",
                                       name="backps")
                nc.tensor.transpose(backps[:], rrec[:], ident_sb[0:128, 0:128])
                r2sb = small.tile([32, 128], BF16, tag="r2b", name="r2b")
                nc.vector.tensor_copy(r2sb[:], backps[:])
                for g in range(NG):
                    rbc = aux_pool.tile([128, 512], F32, tag="aux",
                                        name="rbc")
                    for b in range(4):
                        s = g * 4 + b
                        nc.tensor.matmul(
                            rbc[:, b * 128:(b + 1) * 128],
                            sel_sb[:, s * 128:(s + 1) * 128],
                            r2sb[0:32, :],
                            start=True, stop=True)
                    apply_fn(g, rbc)

            def apply1(p):
                def fn(g, rbc):
                    for hh in range(2):
                        r0 = hh * 64
                        sl = slice(g * 512, (g + 1) * 512)
                        nc.vector.tensor_mul(zT[p][r0:r0 + 64, sl],
                                             zT[p][r0:r0 + 64, sl],
                                             rbc[r0:r0 + 64, :])
                return fn

            # pass-1: pair 0 was woven into the projections (phase A);
            # normalize it, then run pair 1 (k2 pair-0 slices woven in) and
            # normalize that.
            normalize(0, dten_p0, apply1(0))
            dten1 = attn_pass(1, kpair[1], (vaug_sb[2], vaug_sb[3]),
                              0, zT[1],
                              between_g=[k2_slice(0, n) for n in range(4)])

            # ---- pass 2 (MA), e-blocks spread across the j-iterations ----
            def make_ework(p):
                def ework(g):
                    box = {}

                    def sub_th():
                        et = etmp.tile([128, 512], BF16, tag="et", name="et")
                        nc.vector.tensor_sub(
                            et[:],
                            xpair[p][:, g * 512 + 1:(g + 1) * 512 + 1],
                            zT[p][:, g * 512:(g + 1) * 512])
                        box["et"] = et
                        for hh in range(2):
                            ones_ap = eaug_sb[2 * p + hh][:].rearrange(
                                "q (j c) -> q j c", c=128)[:, 4 * g:4 * g + 4,
                                                           64:65]
                            nc.sync.dma_start(ones_ap, onesp[:])

                    def tp_th(jj):
                        def th():
                            j = 4 * g + jj
                            tp = py_pool.tile([128, 128], BF16, tag="py",
                                              name="tp")
                            nc.tensor.transpose(
                                tp[:], box["et"][:, jj * 128:(jj + 1) * 128],
                                ident_sb[:])
                            for hh in range(2):
                                nc.vector.tensor_copy(
                                    eaug_sb[2 * p + hh][:,
                                                        j * 128:j * 128 + 64],
                                    tp[:, hh * 64:hh * 64 + 64])
                        return th

                    return [sub_th] + [tp_th(jj) for jj in range(4)]
                return ework

            def apply2(p):
                def fn(g, rbc):
                    tmp = small.tile([128, 512], BF16, tag="tmp", name="tmp")
                    wd = 512 if g < NG - 1 else 511
                    for hh in range(2):
                        r0 = hh * 64
                        sl = slice(g * 512, (g + 1) * 512)
                        nc.vector.tensor_mul(tmp[r0:r0 + 64, :],
                                             y2un[p][r0:r0 + 64, sl],
                                             rbc[r0:r0 + 64, :])
                        dst = zT[p][r0:r0 + 64,
                                    g * 512 + 1:g * 512 + 1 + wd]
                        nc.vector.tensor_add(dst, dst,
                                             tmp[r0:r0 + 64, 0:wd])
                return fn

            nc.sync.dma_start(wp_sb[:], wp[:])
            dten2a = attn_pass(0, k2pair[0], (eaug_sb[0], eaug_sb[1]),
                               1, y2un[0], ework=make_ework(0),
                               between_g=[k2_slice(1, n) for n in range(4)])
            normalize(1, dten1, apply1(1))
            bg = [lambda: normalize(0, dten2a, apply2(0))] + \
                 [(lambda: None) for _ in range(3)]
            dten2b = attn_pass(1, k2pair[1], (eaug_sb[2], eaug_sb[3]),
                               1, y2un[1], ework=make_ework(1),
                               between_g=bg)
            normalize(1, dten2b, apply2(1))

            # ---- out projection: outT[cb] = sum_cc Wp[cc,cb].T @ zT[cc] ----
            actx.close()
            with tc.tile_pool(name="po", bufs=2, space="PSUM") as po, \
                 tc.tile_pool(name="ost", bufs=2) as ost:
                for cb in range(8):
                    acc = po.tile([128, T], F32, tag="po", name="po")
                    for cc in range(NP):
                        for n in range(4):
                            nc.tensor.matmul(
                                acc[:, n * 512:(n + 1) * 512],
                                wp_sb[:, cc * C + cb * 128:
                                      cc * C + cb * 128 + 128],
                                zT[cc][:, n * 512:(n + 1) * 512],
                                start=(cc == 0), stop=(cc == 1))
                    stg = ost.tile([128, T], BF16, tag="stg", name="stg")
                    for q in range(4):
                        sl = slice(q * 512, (q + 1) * 512)
                        nc.vector.tensor_copy(stg[:, sl], acc[:, sl])
                        nc.sync.dma_start(outT[cb * 128:(cb + 1) * 128, sl],
                                          stg[:, sl])

    nc.compile()
    return nc


def _get_built():
    global _BUILT
    if _BUILT is None:
        _BUILT = _build()
    return _BUILT


def _prep_core(x, W_attn, b_attn, W_k2, b_k2, W_proj, core):
    bf16 = ml_dtypes.bfloat16
    b, hg = core // 4, core % 4
    cs = hg * DG
    xb = np.asarray(x[b], dtype=np.float32)
    # roll channels so this core's pair rows land at xT[0:256]
    xTr = np.ascontiguousarray(np.roll(xb.T, -cs, axis=0)).astype(bf16)
    xh = xb[:, cs:cs + DG]

    va = np.zeros((HPC, NT, 128, 128), np.float32)
    for h in range(HPC):
        va[h, :, :, :64] = xh[:, h * D:(h + 1) * D].reshape(NT, 128, D)
        va[h, :, :, 64] = 1.0
    vaug = va.transpose(0, 2, 1, 3).reshape(HPC * 128, NT * 128).astype(bf16)

    def wslice(Wfull, c0):
        Wr = np.roll(Wfull, -cs, axis=0)   # match the x-row roll
        return np.ascontiguousarray(
            Wr[:, c0:c0 + DG].reshape(8, 128, DG).transpose(1, 0, 2)
            .reshape(128, 8 * DG)).astype(bf16)

    wq = wslice(W_attn, cs)
    wk = wslice(W_attn, C + cs)
    wk2 = wslice(W_k2, cs)
    wp = np.ascontiguousarray(
        W_proj[cs:cs + DG, :].reshape(2, 128, C).transpose(1, 0, 2)
        .reshape(128, 2 * C)).astype(bf16)

    bqv = np.ascontiguousarray(b_attn[cs:cs + DG].reshape(NP, 128).T)
    bkv = np.ascontiguousarray(b_attn[C + cs:C + cs + DG].reshape(NP, 128).T)
    bk2v = np.ascontiguousarray(b_k2[cs:cs + DG].reshape(NP, 128).T)

    return dict(xT=xTr, vaug=vaug, wq=wq, wk=wk, wk2=wk2, wp=wp,
                bq=bqv.astype(np.float32), bk=bkv.astype(np.float32),
                bk2=bk2v.astype(np.float32))


def _consts():
    bf16 = ml_dtypes.bfloat16
    mask = np.ones((128, 128), np.float32)
    for kj in range(1, 128):
        mask[kj, :kj] = 0.0
    mask2 = np.concatenate([mask, mask], axis=1).astype(bf16)
    identb = np.eye(128, dtype=np.float32).astype(bf16)
    ones4 = np.ones((128, 4, 1), bf16)
    sel32 = np.zeros((32, 16 * 128), np.float32)
    for g in range(4):
        for b in range(4):
            s = g * 4 + b
            for hh in range(2):
                i = hh * 16 + b * 4 + g
                sel32[i, s * 128 + hh * 64:s * 128 + (hh + 1) * 64] = 1.0
    return dict(mask2=mask2, identb=identb, ones4=ones4,
                sel32=sel32.astype(bf16))


def kernel(x, W_attn, b_attn, W_k2, b_k2, W_proj, b_proj):
    x = np.asarray(x, np.float32)
    W_attn = np.asarray(W_attn, np.float32)
    b_attn = np.asarray(b_attn, np.float32)
    W_k2 = np.asarray(W_k2, np.float32)
    b_k2 = np.asarray(b_k2, np.float32)
    W_proj = np.asarray(W_proj, np.float32)
    b_proj = np.asarray(b_proj, np.float32)

    cst = _consts()
    in_maps = []
    for core in range(8):
        m = _prep_core(x, W_attn, b_attn, W_k2, b_k2, W_proj, core)
        m.update(cst)
        in_maps.append(m)

    nc = _get_built()
    res = run_bass_kernel_spmd(nc, in_maps, list(range(8)))

    out = np.zeros((B, T, C), np.float32)
    for core in range(8):
        out[core // 4] += res.results[core]["outT"].astype(np.float32).T
    out += 2.0 * b_proj
    return out



# revision 11
# speedup vs baseline: 1.1481x; 1.1481x over previous
"""Trainium2 Bass kernel for CausalSelfAttentionARMA (eval forward).

Sharding: 8 cores = 2 batches x 4 head-groups (4 heads each, d=64). Each core
computes its (batch, head-group) shard end-to-end and returns a transposed
partial output [C, T] in bf16; the host sums partials per batch and adds
2*b_proj.

v4 design:
  - x resident in SBUF (8x[128,2056] bf16), big DMAs on 4 queues; the
    q/k/k2 projections read it directly (no re-streaming from HBM).
  - AV matmuls in fp8e4 with MatmulPerfMode.DoubleRow: pairs of 128-key
    blocks contracted per matmul (virtual K=256); exp output written to fp8
    pair tiles [128, 2, 2heads*512]. Ones-column in the fp8 stationary
    produces softmax denominators in psum row 64.
  - software-pipelined unit stream: AV(jp) is emitted between the two QK
    matmul halves of jp+1 so the ACT queue (exp backbone) never waits on
    PE; the per-group normalize chain is deferred past the next group's
    first unit.
  - per-group fused normalization: vector.reciprocal on the psum den rows,
    f32 selector-matmul broadcast into a py-pool psum tile, one DVE copy
    to SBUF, then fused psum*rbc multiplies write normalized y into zT.
    Pass 2 adds shifted y2 with a one-column fixup in the next chain.
  - e = vshift - y computed from resident x, PE-transposed into fp8 eaug
    tiles; e-prep and k2-projection subunits are woven into the pass
    unit streams.
"""
import numpy as np
import ml_dtypes

import concourse.bass as bass
import concourse.bacc as bacc
import concourse.tile as tile
from concourse import mybir
from concourse.bass_utils import run_bass_kernel_spmd

F32 = mybir.dt.float32
BF16 = mybir.dt.bfloat16
FP8 = mybir.dt.float8e4
E4NP = ml_dtypes.float8_e4m3

B, T, C = 2, 2048, 1024
H, D = 16, 64
HPC = 4                # heads per core
DG = HPC * D           # 256
NT = T // 128          # 16 key tiles
NG = 4                 # q groups of 512
NP = 2                 # head pairs per core
JW = 128               # stationary row per key tile: [ones, 63*0, 64 v]

_BUILT = None


def _build():
    nc = bacc.Bacc("TRN2", target_bir_lowering=False, debug=False,
                   num_devices=8)

    xT = nc.declare_dram_parameter("xT", [C, T], BF16, isOutput=False)
    vaugb = nc.declare_dram_parameter("vaugb", [HPC * 128, NT * JW], BF16,
                                      isOutput=False)
    wq = nc.declare_dram_parameter("wq", [128, 8 * DG], BF16, isOutput=False)
    wk = nc.declare_dram_parameter("wk", [128, 8 * DG], BF16, isOutput=False)
    wk2 = nc.declare_dram_parameter("wk2", [128, 8 * DG], BF16, isOutput=False)
    wp = nc.declare_dram_parameter("wp", [128, 2 * C], BF16, isOutput=False)
    bq = nc.declare_dram_parameter("bq", [128, NP], F32, isOutput=False)
    bk = nc.declare_dram_parameter("bk", [128, NP], F32, isOutput=False)
    bk2 = nc.declare_dram_parameter("bk2", [128, NP], F32, isOutput=False)
    maskp = nc.declare_dram_parameter("mask2", [128, 256], BF16, isOutput=False)
    identp = nc.declare_dram_parameter("identb", [128, 128], BF16,
                                       isOutput=False)
    outT = nc.declare_dram_parameter("outT", [C, T], BF16, isOutput=True)

    with tile.TileContext(nc) as tc:
        import contextlib
        with contextlib.ExitStack() as ctx:
            const = ctx.enter_context(tc.tile_pool(name="const", bufs=1))
            persist = ctx.enter_context(tc.tile_pool(name="persist", bufs=1))
            small = ctx.enter_context(tc.tile_pool(name="small", bufs=3))
            expp = ctx.enter_context(tc.tile_pool(name="expp", bufs=4))
            etmp = ctx.enter_context(tc.tile_pool(name="etmp", bufs=2))

            # ---- small consts on sync (cheap, first) ----
            bias_sb = {}
            for nm, par in (("bq", bq), ("bk", bk), ("bk2", bk2)):
                t = const.tile([128, NP], F32, tag=nm, name=nm)
                nc.sync.dma_start(t[:], par[:])
                bias_sb[nm] = t
            mask_sb = const.tile([128, 256], BF16, tag="mask")
            nc.sync.dma_start(mask_sb[:], maskp[:])
            ident_sb = const.tile([128, 128], BF16, tag="ident")
            nc.sync.dma_start(ident_sb[:], identp[:])

            # exp act-table preload: tiny dummy exp during preamble DMAs
            scratch = const.tile([1, 2], F32, tag="scr")
            nc.scalar.activation(scratch[:], bias_sb["bq"][0:1, :],
                                 mybir.ActivationFunctionType.Exp, scale=1.0)

            # ---- big input DMAs spread over 4 queues ----
            wq_sb = const.tile([128, 8 * DG], BF16, tag="wq")
            wk_sb = const.tile([128, 8 * DG], BF16, tag="wk")
            wk2_sb = const.tile([128, 8 * DG], BF16, tag="wk2")
            wp_sb = const.tile([128, 2 * C], BF16, tag="wp")
            nc.sync.dma_start(wq_sb[:], wq[:])
            nc.gpsimd.dma_start(wk_sb[:], wk[:])

            xsb = [persist.tile([128, T + 8], BF16, tag=f"x{c}", name=f"x{c}")
                   for c in range(8)]
            vaug_sb = [persist.tile([128, NT * JW], BF16, tag=f"vaug{h}",
                                    name=f"vaug{h}") for h in range(HPC)]
            eaug_sb = [persist.tile([128, NT * JW], BF16, tag=f"eaug{h}",
                                    name=f"eaug{h}") for h in range(HPC)]

            qs = [nc.sync, nc.gpsimd, nc.scalar]
            nc.scalar.dma_start(vaug_sb[0][:], vaugb[0:128, :])
            nc.scalar.dma_start(vaug_sb[1][:], vaugb[128:256, :])
            for c in range(8):
                qs[c % 3].dma_start(xsb[c][:, 0:1024],
                                    xT[c * 128:(c + 1) * 128, 0:1024])
            for c in range(8):
                qs[(c + 1) % 3].dma_start(xsb[c][:, 1024:2048],
                                          xT[c * 128:(c + 1) * 128,
                                             1024:2048])
            nc.scalar.dma_start(vaug_sb[2][:], vaugb[256:384, :])
            nc.scalar.dma_start(vaug_sb[3][:], vaugb[384:512, :])
            nc.sync.dma_start(wk2_sb[:], wk2[:])
            nc.gpsimd.dma_start(wp_sb[:], wp[:])
            for p in range(NP):
                nc.gpsimd.memset(xsb[p][:, T:T + 8], 0.0)
            # eaug: ones at col 0, zeros at cols 1:32 of each key-tile row
            for h in range(HPC):
                ev = eaug_sb[h][:].rearrange("k (j c) -> k j c", c=JW)
                nc.gpsimd.memset(ev[:, :, 0:1], 1.0)
                nc.gpsimd.memset(ev[:, :, 1:64], 0.0)

            qpair = [persist.tile([128, T + 8], BF16, tag=f"q{p}", name=f"q{p}")
                     for p in range(NP)]
            kpair = [persist.tile([128, T], BF16, tag=f"k{p}", name=f"k{p}")
                     for p in range(NP)]
            k2pair = [persist.tile([128, T], BF16, tag=f"k2{p}", name=f"k2{p}")
                      for p in range(NP)]
            zT = [persist.tile([128, T], BF16, tag=f"zT{p}", name=f"zT{p}")
                  for p in range(NP)]

            # =========== pass unit machinery ===========
            def make_av(py_pool, sts, p, stats, g, j):
                """bf16 AV for key block j of group g. Stationary layout per
                j: col 0 = ones (den -> psum row 0), cols 64:128 = v
                (y -> psum rows 64:128)."""
                def av():
                    st = sts[g]
                    if j == 0:
                        st["py"] = [py_pool.tile([128, 512], F32, tag="py",
                                                 name="py") for _ in range(2)]
                    col0 = max(0, (j - 4 * g) * 128)
                    texp = st.pop(f"texp{j}")
                    tv = texp[:].rearrange("k (h q) -> k h q", h=2)
                    for hh in range(2):
                        sv = stats[hh][:].rearrange("k (j c) -> k j c", c=JW)
                        nc.tensor.matmul(
                            st["py"][hh][0:128, col0:512],
                            sv[:, j, 0:128],
                            tv[:, hh, col0:512],
                            start=(j == 0), stop=(j == 4 * g + 3),
                            skip_group_check=True)
                return av

            def make_unit(ps_pool, sts, p, keysT, qoff, g, j, av_mid):
                """texp alloc + QK(j) with pending AV between the two head
                matmuls + exp + mask for key block j of group g."""
                def u():
                    st = sts[g]
                    col0 = max(0, (j - 4 * g) * 128)
                    texp = expp.tile([128, 1024], BF16, tag="texp",
                                     name="texp")
                    st[f"texp{j}"] = texp
                    tv = texp[:].rearrange("k (h q) -> k h q", h=2)
                    ps = ps_pool.tile([128, 1024], F32, tag="ps", name="ps")
                    for hh in range(2):
                        nc.tensor.matmul(
                            ps[:, hh * 512 + col0:(hh + 1) * 512],
                            keysT[hh * 64:(hh + 1) * 64,
                                  j * 128:(j + 1) * 128],
                            qpair[p][hh * 64:(hh + 1) * 64,
                                     qoff + g * 512 + col0:
                                     qoff + (g + 1) * 512],
                            start=True, stop=True,
                            tile_position=(hh * 64, 0))
                        if hh == 0 and av_mid is not None:
                            av_mid()
                    if col0 == 0:
                        nc.scalar.activation(
                            texp[:], ps[:],
                            mybir.ActivationFunctionType.Exp, scale=0.125)
                    else:
                        pv = ps[:].rearrange("k (h q) -> k h q", h=2)
                        nc.scalar.activation(
                            tv[:, :, col0:512], pv[:, :, col0:512],
                            mybir.ActivationFunctionType.Exp, scale=0.125)
                    if j >= 4 * g:
                        mv = mask_sb[:].rearrange("k (h w) -> k h w", w=128)
                        nc.vector.tensor_mul(tv[:, :, col0:col0 + 128],
                                             tv[:, :, col0:col0 + 128], mv)
                return u

            def make_chain(sts, pst, p, g, pass_no):
                """Normalize group g: approx-recip of psum den row 0, gpsimd
                partition broadcast, fused psum*rbc multiplies into zT."""
                def chain():
                    pyA, pyB = sts[g]["py"]
                    ra = small.tile([1, 512], F32, tag="ra", name="ra")
                    rb = small.tile([1, 512], F32, tag="rb", name="rb")
                    nc.vector.reciprocal_approx_fast(ra[:], pyA[0:1, :])
                    nc.vector.reciprocal_approx_fast(rb[:], pyB[0:1, :])
                    rbcA = small.tile([64, 512], F32, tag="bca", name="bca")
                    rbcB = small.tile([64, 512], F32, tag="bcb", name="bcb")
                    nc.gpsimd.partition_broadcast(rbcA[:], ra[:])
                    nc.gpsimd.partition_broadcast(rbcB[:], rb[:])
                    gsl = slice(g * 512, (g + 1) * 512)
                    if pass_no == 1:
                        nc.vector.tensor_mul(zT[p][0:64, gsl], pyA[64:128, :],
                                             rbcA[:])
                        nc.vector.tensor_mul(zT[p][64:128, gsl], pyB[64:128, :],
                                             rbcB[:])
                    else:
                        tmp = small.tile([128, 512], BF16, tag="tmp",
                                         name="tmp")
                        nc.vector.tensor_mul(tmp[0:64, :], pyA[64:128, :],
                                             rbcA[:])
                        nc.vector.tensor_mul(tmp[64:128, :], pyB[64:128, :],
                                             rbcB[:])
                        if g >= 1 and "ptmp" in pst:
                            cc = slice(g * 512, g * 512 + 1)
                            nc.vector.tensor_add(zT[p][:, cc], zT[p][:, cc],
                                                 pst["ptmp"][:, 511:512])
                        dst = zT[p][:, g * 512 + 1:g * 512 + 512]
                        nc.vector.tensor_add(dst, dst, tmp[:, 0:511])
                        pst["ptmp"] = tmp
                return chain

            def pass_chunks(ps_pool, py_pool, p, keysT, stats, qoff, pass_no):
                """Per-group unit chunks (software-pipelined): chunk[g] may
                only run once keysT/qpair cols < 512*(g+1) are final."""
                sts = {g: {} for g in range(NG)}
                pst = {}
                chunks = []
                pend_av = None
                pend_chain = None
                for g in range(NG):
                    cu = []
                    for j in range(4 * g + 4):
                        cu.append(make_unit(ps_pool, sts, p, keysT, qoff, g,
                                            j, pend_av))
                        pend_av = make_av(py_pool, sts, p, stats, g, j)
                        if j == 0 and pend_chain is not None:
                            cu.append(pend_chain)
                            pend_chain = None
                    pend_chain = make_chain(sts, pst, p, g, pass_no)
                    chunks.append(cu)
                chunks.append([pend_av, pend_chain])
                return chunks

            # e-prep subunits for pass 2 of pair p, key-group gp
            def eprep_units(aux_pool, p, gp):
                box = {}

                def sub_u():
                    et = etmp.tile([128, 512], BF16, tag="et", name="et")
                    nc.vector.tensor_sub(
                        et[:],
                        xsb[p][:, gp * 512 + 1:(gp + 1) * 512 + 1],
                        zT[p][:, gp * 512:(gp + 1) * 512])
                    box["et"] = et

                def tp_u(jj):
                    def th():
                        j = 4 * gp + jj
                        tp = aux_pool.tile([128, 512], BF16, tag="aux",
                                           name="tp")
                        nc.tensor.transpose(
                            tp[:, 0:128],
                            box["et"][:, jj * 128:(jj + 1) * 128],
                            ident_sb[:])
                        for hh in range(2):
                            nc.vector.tensor_copy(
                                eaug_sb[2 * p + hh][:, j * JW + 64:
                                                    j * JW + 128],
                                tp[:, hh * 64:hh * 64 + 64])
                    return th

                return [sub_u] + [tp_u(jj) for jj in range(4)]

            # k2 projection subunits for pair pp, t-slice n (reads resident x)
            def k2_units(aux_pool, pp, n):
                box = {}

                def mm_u(ci):
                    def th():
                        if ci == 0:
                            box["acc"] = aux_pool.tile([128, 512], F32,
                                                       tag="aux", name="k2acc")
                        for c in (2 * ci, 2 * ci + 1):
                            nc.tensor.matmul(
                                box["acc"][:],
                                wk2_sb[:, c * DG + pp * 128:
                                       c * DG + pp * 128 + 128],
                                xsb[c][:, n * 512:(n + 1) * 512],
                                start=(c == 0), stop=(c == 7),
                                skip_group_check=True)
                    return th

                def bias_u():
                    nc.vector.tensor_scalar_add(
                        k2pair[pp][:, n * 512:(n + 1) * 512],
                        box["acc"][:], bias_sb["bk2"][:, pp:pp + 1])

                return [mm_u(ci) for ci in range(4)] + [bias_u]

            def run_chunks(chunks, extras):
                for g in range(NG + 1):
                    prim = chunks[g]
                    sec = list(extras.get(g, []))
                    for u in prim:
                        u()
                        if sec:
                            sec.pop(0)()
                    while sec:
                        sec.pop(0)()

            # ============ phase A: q,k projections + woven P1p0 ============
            from collections import deque
            wv = deque()

            def weave():
                if wv:
                    wv.popleft()()

            with tc.tile_pool(name="pproj", bufs=1, space="PSUM") as pproj, \
                 tc.tile_pool(name="psA", bufs=2, space="PSUM") as psA, \
                 tc.tile_pool(name="pyA", bufs=3, space="PSUM") as pyA:
                p0chunks = pass_chunks(psA, pyA, 0, kpair[0],
                                       (vaug_sb[0], vaug_sb[1]), 0, 1)
                plan = [(wq_sb, qpair, "bq"), (wk_sb, kpair, "bk")]
                for n in range(NG):
                    for pp in range(NP):
                        for w_sb, dsts, bnm in plan:
                            acc = pproj.tile([128, 512], F32, tag="acc",
                                             name="acc")
                            for c in range(8):
                                nc.tensor.matmul(
                                    acc[:],
                                    w_sb[:, c * DG + pp * 128:
                                         c * DG + pp * 128 + 128],
                                    xsb[c][:, n * 512:(n + 1) * 512],
                                    start=(c == 0), stop=(c == 7))
                                weave()
                            nc.vector.tensor_scalar_add(
                                dsts[pp][:, n * 512:(n + 1) * 512],
                                acc[:], bias_sb[bnm][:, pp:pp + 1])
                            weave()
                        if pp == 0:
                            wv.extend(p0chunks[n])
                while wv:
                    wv.popleft()()
                for u in p0chunks[NG]:
                    u()

            # pad col for shifted q (col T = col T-1; value discarded)
            for p in range(NP):
                nc.vector.tensor_copy(qpair[p][:, T:T + 1],
                                      qpair[p][:, T - 1:T])

            # ============ phase B ============
            actx = contextlib.ExitStack()
            ps_pool = actx.enter_context(
                tc.tile_pool(name="ps", bufs=2, space="PSUM"))
            py_pool = actx.enter_context(
                tc.tile_pool(name="py", bufs=3, space="PSUM"))
            aux_pool = actx.enter_context(
                tc.tile_pool(name="aux", bufs=1, space="PSUM"))

            # P1p1: weave k2(p0) + eprep(p0, 0) late
            extras = {g: k2_units(aux_pool, 0, g) for g in range(NG)}
            extras[2] = extras[2] + eprep_units(aux_pool, 0, 0)
            run_chunks(pass_chunks(ps_pool, py_pool, 1, kpair[1],
                                   (vaug_sb[2], vaug_sb[3]), 0, 1), extras)

            # P2p0: weave k2(p1), eprep(p0, g) g=1..3, eprep(p1, 0) late
            extras = {g: k2_units(aux_pool, 1, g) for g in range(NG)}
            for gp in range(1, NG):
                extras[gp - 1] = extras[gp - 1] + eprep_units(aux_pool, 0, gp)
            extras[3] = extras[3] + eprep_units(aux_pool, 1, 0)
            run_chunks(pass_chunks(ps_pool, py_pool, 0, k2pair[0],
                                   (eaug_sb[0], eaug_sb[1]), 1, 2), extras)

            # P2p1: weave eprep(p1, g) g=1..3
            extras = {}
            for gp in range(1, NG):
                extras[gp - 1] = eprep_units(aux_pool, 1, gp)
            run_chunks(pass_chunks(ps_pool, py_pool, 1, k2pair[1],
                                   (eaug_sb[2], eaug_sb[3]), 1, 2), extras)

            # ============ tail: out projection ============
            actx.close()
            with tc.tile_pool(name="po", bufs=3, space="PSUM") as po, \
                 tc.tile_pool(name="ost", bufs=4) as ost:
                for cb in range(8):
                    accs = [po.tile([128, 1024], F32, tag="po", name="po")
                            for _ in range(2)]
                    for cc in range(NP):
                        for half in range(2):
                            for nn in range(2):
                                sl = slice(half * 1024 + nn * 512,
                                           half * 1024 + (nn + 1) * 512)
                                nc.tensor.matmul(
                                    accs[half][:, nn * 512:(nn + 1) * 512],
                                    wp_sb[:, cc * C + cb * 128:
                                          cc * C + cb * 128 + 128],
                                    zT[cc][:, sl],
                                    start=(cc == 0), stop=(cc == 1))
                    for half in range(2):
                        stg = ost.tile([128, 1024], BF16, tag="stg",
                                       name="stg")
                        if half == 0:
                            nc.vector.tensor_copy(stg[:], accs[half][:])
                            nc.sync.dma_start(
                                outT[cb * 128:(cb + 1) * 128, 0:1024], stg[:])
                        else:
                            nc.scalar.copy(stg[:], accs[half][:])
                            nc.gpsimd.dma_start(
                                outT[cb * 128:(cb + 1) * 128, 1024:2048],
                                stg[:])

    nc.compile()
    return nc


def _get_built():
    global _BUILT
    if _BUILT is None:
        _BUILT = _build()
    return _BUILT


def _prep_core(x, W_attn, b_attn, W_k2, b_k2, W_proj, core):
    bf16 = ml_dtypes.bfloat16
    b, hg = core // 4, core % 4
    cs = hg * DG
    xb = np.asarray(x[b], dtype=np.float32)
    # roll channels so this core's pair rows land at xT[0:256]
    xTr = np.ascontiguousarray(np.roll(xb.T, -cs, axis=0)).astype(bf16)
    xh = xb[:, cs:cs + DG]

    # vaugb: per head, [128 ki, NT tiles * JW]: col 0 = ones, 64:128 = v
    va = np.zeros((HPC, 128, NT, JW), np.float32)
    for h in range(HPC):
        va[h, :, :, 0] = 1.0
        va[h, :, :, 64:128] = xh[:, h * D:(h + 1) * D].reshape(NT, 128, D) \
            .transpose(1, 0, 2)
    vaug = np.ascontiguousarray(va.reshape(HPC * 128, NT * JW)).astype(bf16)

    def wslice(Wfull, c0):
        Wr = np.roll(Wfull, -cs, axis=0)   # match the x-row roll
        return np.ascontiguousarray(
            Wr[:, c0:c0 + DG].reshape(8, 128, DG).transpose(1, 0, 2)
            .reshape(128, 8 * DG)).astype(bf16)

    wq = wslice(W_attn, cs)
    wk = wslice(W_attn, C + cs)
    wk2 = wslice(W_k2, cs)
    wp = np.ascontiguousarray(
        W_proj[cs:cs + DG, :].reshape(2, 128, C).transpose(1, 0, 2)
        .reshape(128, 2 * C)).astype(bf16)

    bqv = np.ascontiguousarray(b_attn[cs:cs + DG].reshape(NP, 128).T)
    bkv = np.ascontiguousarray(b_attn[C + cs:C + cs + DG].reshape(NP, 128).T)
    bk2v = np.ascontiguousarray(b_k2[cs:cs + DG].reshape(NP, 128).T)

    return dict(xT=xTr, vaugb=vaug, wq=wq, wk=wk, wk2=wk2, wp=wp,
                bq=bqv.astype(np.float32), bk=bkv.astype(np.float32),
                bk2=bk2v.astype(np.float32))


def _consts():
    bf16 = ml_dtypes.bfloat16
    mask = np.ones((128, 128), np.float32)
    for kj in range(1, 128):
        mask[kj, :kj] = 0.0
    mask2 = np.concatenate([mask, mask], axis=1).astype(bf16)
    identb = np.eye(128, dtype=np.float32).astype(bf16)
    return dict(mask2=mask2, identb=identb)


def kernel(x, W_attn, b_attn, W_k2, b_k2, W_proj, b_proj):
    x = np.asarray(x, np.float32)
    W_attn = np.asarray(W_attn, np.float32)
    b_attn = np.asarray(b_attn, np.float32)
    W_k2 = np.asarray(W_k2, np.float32)
    b_k2 = np.asarray(b_k2, np.float32)
    W_proj = np.asarray(W_proj, np.float32)
    b_proj = np.asarray(b_proj, np.float32)

    cst = _consts()
    in_maps = []
    for core in range(8):
        m = _prep_core(x, W_attn, b_attn, W_k2, b_k2, W_proj, core)
        m.update(cst)
        in_maps.append(m)

    nc = _get_built()
    res = run_bass_kernel_spmd(nc, in_maps, list(range(8)))

    out = np.zeros((B, T, C), np.float32)
    for core in range(8):
        out[core // 4] += res.results[core]["outT"].astype(np.float32).T
    out += 2.0 * b_proj
    return out


# revision 14
# speedup vs baseline: 1.1500x; 1.0017x over previous
"""Trainium2 Bass kernel for CausalSelfAttentionARMA (eval forward).

Sharding: 8 cores = 2 batches x 4 head-groups (4 heads each, d=64). Each core
computes its (batch, head-group) shard end-to-end and returns a transposed
partial output [C, T] in bf16; the host sums partials per batch and adds
2*b_proj.

v4 design:
  - x resident in SBUF (8x[128,2056] bf16), big DMAs on 4 queues; the
    q/k/k2 projections read it directly (no re-streaming from HBM).
  - AV matmuls in fp8e4 with MatmulPerfMode.DoubleRow: pairs of 128-key
    blocks contracted per matmul (virtual K=256); exp output written to fp8
    pair tiles [128, 2, 2heads*512]. Ones-column in the fp8 stationary
    produces softmax denominators in psum row 64.
  - software-pipelined unit stream: AV(jp) is emitted between the two QK
    matmul halves of jp+1 so the ACT queue (exp backbone) never waits on
    PE; the per-group normalize chain is deferred past the next group's
    first unit.
  - per-group fused normalization: vector.reciprocal on the psum den rows,
    f32 selector-matmul broadcast into a py-pool psum tile, one DVE copy
    to SBUF, then fused psum*rbc multiplies write normalized y into zT.
    Pass 2 adds shifted y2 with a one-column fixup in the next chain.
  - e = vshift - y computed from resident x, PE-transposed into fp8 eaug
    tiles; e-prep and k2-projection subunits are woven into the pass
    unit streams.
"""
import numpy as np
import ml_dtypes

import concourse.bass as bass
import concourse.bacc as bacc
import concourse.tile as tile
from concourse import mybir
from concourse.bass_utils import run_bass_kernel_spmd

F32 = mybir.dt.float32
BF16 = mybir.dt.bfloat16
FP8 = mybir.dt.float8e4
E4NP = ml_dtypes.float8_e4m3

B, T, C = 2, 2048, 1024
H, D = 16, 64
HPC = 4                # heads per core
DG = HPC * D           # 256
NT = T // 128          # 16 key tiles
NG = 4                 # q groups of 512
NP = 2                 # head pairs per core
JW = 128               # stationary row per key tile: [ones, 63*0, 64 v]

_BUILT = None


def _build():
    nc = bacc.Bacc("TRN2", target_bir_lowering=False, debug=False,
                   num_devices=8)

    xT = nc.declare_dram_parameter("xT", [C, T], BF16, isOutput=False)
    vaugb = nc.declare_dram_parameter("vaugb", [HPC * 128, NT * JW], BF16,
                                      isOutput=False)
    wq = nc.declare_dram_parameter("wq", [128, 8 * DG], BF16, isOutput=False)
    wk = nc.declare_dram_parameter("wk", [128, 8 * DG], BF16, isOutput=False)
    wk2 = nc.declare_dram_parameter("wk2", [128, 8 * DG], BF16, isOutput=False)
    wp = nc.declare_dram_parameter("wp", [128, 2 * C], BF16, isOutput=False)
    bq = nc.declare_dram_parameter("bq", [128, NP], F32, isOutput=False)
    bk = nc.declare_dram_parameter("bk", [128, NP], F32, isOutput=False)
    bk2 = nc.declare_dram_parameter("bk2", [128, NP], F32, isOutput=False)
    maskp = nc.declare_dram_parameter("mask2", [128, 256], BF16, isOutput=False)
    identp = nc.declare_dram_parameter("identb", [128, 128], BF16,
                                       isOutput=False)
    outT = nc.declare_dram_parameter("outT", [C, T], BF16, isOutput=True)

    with tile.TileContext(nc) as tc:
        import contextlib
        with contextlib.ExitStack() as ctx:
            const = ctx.enter_context(tc.tile_pool(name="const", bufs=1))
            persist = ctx.enter_context(tc.tile_pool(name="persist", bufs=1))
            small = ctx.enter_context(tc.tile_pool(name="small", bufs=3))
            expp = ctx.enter_context(tc.tile_pool(name="expp", bufs=4))
            etmp = ctx.enter_context(tc.tile_pool(name="etmp", bufs=2))

            # ---- small consts on sync (cheap, first) ----
            bias_sb = {}
            for nm, par in (("bq", bq), ("bk", bk), ("bk2", bk2)):
                t = const.tile([128, NP], F32, tag=nm, name=nm)
                nc.sync.dma_start(t[:], par[:])
                bias_sb[nm] = t
            mask_sb = const.tile([128, 256], BF16, tag="mask")
            nc.sync.dma_start(mask_sb[:], maskp[:])
            ident_sb = const.tile([128, 128], BF16, tag="ident")
            nc.sync.dma_start(ident_sb[:], identp[:])

            # exp act-table preload: tiny dummy exp during preamble DMAs
            scratch = const.tile([1, 2], F32, tag="scr")
            nc.scalar.activation(scratch[:], bias_sb["bq"][0:1, :],
                                 mybir.ActivationFunctionType.Exp, scale=1.0)

            # ---- big input DMAs spread over 4 queues ----
            wq_sb = const.tile([128, 8 * DG], BF16, tag="wq")
            wk_sb = const.tile([128, 8 * DG], BF16, tag="wk")
            wk2_sb = const.tile([128, 8 * DG], BF16, tag="wk2")
            wp_sb = const.tile([128, 2 * C], BF16, tag="wp")
            nc.sync.dma_start(wq_sb[:], wq[:])
            nc.gpsimd.dma_start(wk_sb[:], wk[:])

            xsb = [persist.tile([128, T + 8], BF16, tag=f"x{c}", name=f"x{c}")
                   for c in range(8)]
            vaug_sb = [persist.tile([128, NT * JW], BF16, tag=f"vaug{h}",
                                    name=f"vaug{h}") for h in range(HPC)]
            eaug_sb = [persist.tile([128, NT * JW], BF16, tag=f"eaug{h}",
                                    name=f"eaug{h}") for h in range(HPC)]

            qs = [nc.sync, nc.gpsimd, nc.scalar]
            nc.scalar.dma_start(vaug_sb[0][:], vaugb[0:128, :])
            nc.scalar.dma_start(vaug_sb[1][:], vaugb[128:256, :])
            for c in range(8):
                qs[c % 3].dma_start(xsb[c][:, 0:1024],
                                    xT[c * 128:(c + 1) * 128, 0:1024])
            for c in range(8):
                qs[(c + 1) % 3].dma_start(xsb[c][:, 1024:2048],
                                          xT[c * 128:(c + 1) * 128,
                                             1024:2048])
            nc.scalar.dma_start(vaug_sb[2][:], vaugb[256:384, :])
            nc.scalar.dma_start(vaug_sb[3][:], vaugb[384:512, :])
            nc.sync.dma_start(wk2_sb[:], wk2[:])
            nc.gpsimd.dma_start(wp_sb[:], wp[:])
            for p in range(NP):
                nc.gpsimd.memset(xsb[p][:, T:T + 8], 0.0)
            # eaug: ones at col 0, zeros at cols 1:32 of each key-tile row
            for h in range(HPC):
                ev = eaug_sb[h][:].rearrange("k (j c) -> k j c", c=JW)
                nc.gpsimd.memset(ev[:, :, 0:1], 1.0)
                nc.gpsimd.memset(ev[:, :, 1:64], 0.0)

            qpair = [persist.tile([128, T + 8], BF16, tag=f"q{p}", name=f"q{p}")
                     for p in range(NP)]
            kpair = [persist.tile([128, T], BF16, tag=f"k{p}", name=f"k{p}")
                     for p in range(NP)]
            k2pair = [persist.tile([128, T], BF16, tag=f"k2{p}", name=f"k2{p}")
                      for p in range(NP)]
            zT = [persist.tile([128, T], BF16, tag=f"zT{p}", name=f"zT{p}")
                  for p in range(NP)]

            # =========== pass unit machinery ===========
            def make_av(py_pool, sts, p, stats, g, j):
                """bf16 AV for key block j of group g. Stationary layout per
                j: col 0 = ones (den -> psum row 0), cols 64:128 = v
                (y -> psum rows 64:128)."""
                def av():
                    st = sts[g]
                    if j == 0:
                        st["py"] = [py_pool.tile([128, 512], F32, tag="py",
                                                 name="py") for _ in range(2)]
                    col0 = max(0, (j - 4 * g) * 128)
                    texp = st.pop(f"texp{j}")
                    tv = texp[:].rearrange("k (h q) -> k h q", h=2)
                    for hh in range(2):
                        sv = stats[hh][:].rearrange("k (j c) -> k j c", c=JW)
                        nc.tensor.matmul(
                            st["py"][hh][0:128, col0:512],
                            sv[:, j, 0:128],
                            tv[:, hh, col0:512],
                            start=(j == 0), stop=(j == 4 * g + 3),
                            skip_group_check=True)
                return av

            def make_unit(ps_pool, sts, p, keysT, qoff, g, j, av_mid):
                """texp alloc + QK(j) with pending AV between the two head
                matmuls + exp + mask for key block j of group g."""
                def u():
                    st = sts[g]
                    col0 = max(0, (j - 4 * g) * 128)
                    texp = expp.tile([128, 1024], BF16, tag="texp",
                                     name="texp")
                    st[f"texp{j}"] = texp
                    tv = texp[:].rearrange("k (h q) -> k h q", h=2)
                    ps = ps_pool.tile([128, 1024], F32, tag="ps", name="ps")
                    for hh in range(2):
                        nc.tensor.matmul(
                            ps[:, hh * 512 + col0:(hh + 1) * 512],
                            keysT[hh * 64:(hh + 1) * 64,
                                  j * 128:(j + 1) * 128],
                            qpair[p][hh * 64:(hh + 1) * 64,
                                     qoff + g * 512 + col0:
                                     qoff + (g + 1) * 512],
                            start=True, stop=True,
                            tile_position=(hh * 64, 0))
                        if hh == 0 and av_mid is not None:
                            av_mid()
                    if col0 == 0:
                        nc.scalar.activation(
                            texp[:], ps[:],
                            mybir.ActivationFunctionType.Exp, scale=0.125)
                    else:
                        pv = ps[:].rearrange("k (h q) -> k h q", h=2)
                        nc.scalar.activation(
                            tv[:, :, col0:512], pv[:, :, col0:512],
                            mybir.ActivationFunctionType.Exp, scale=0.125)
                    if j >= 4 * g:
                        mv = mask_sb[:].rearrange("k (h w) -> k h w", w=128)
                        nc.vector.tensor_mul(tv[:, :, col0:col0 + 128],
                                             tv[:, :, col0:col0 + 128], mv)
                return u

            def make_chain(sts, pst, p, g, pass_no):
                """Normalize group g: approx-recip of psum den row 0, gpsimd
                partition broadcast, fused psum*rbc multiplies into zT."""
                def chain():
                    pyA, pyB = sts[g]["py"]
                    # copy y rows out + recip den rows so py slots free fast
                    cpA = small.tile([64, 512], F32, tag="cpa", name="cpa")
                    cpB = small.tile([64, 512], F32, tag="cpb", name="cpb")
                    nc.vector.tensor_copy(cpA[:], pyA[64:128, :])
                    nc.vector.tensor_copy(cpB[:], pyB[64:128, :])
                    ra = small.tile([1, 512], F32, tag="ra", name="ra")
                    rb = small.tile([1, 512], F32, tag="rb", name="rb")
                    nc.vector.reciprocal_approx_fast(ra[:], pyA[0:1, :])
                    nc.vector.reciprocal_approx_fast(rb[:], pyB[0:1, :])
                    rbcA = small.tile([64, 512], F32, tag="bca", name="bca")
                    rbcB = small.tile([64, 512], F32, tag="bcb", name="bcb")
                    nc.gpsimd.partition_broadcast(rbcA[:], ra[:])
                    nc.gpsimd.partition_broadcast(rbcB[:], rb[:])
                    gsl = slice(g * 512, (g + 1) * 512)
                    if pass_no == 1:
                        nc.vector.tensor_mul(zT[p][0:64, gsl], cpA[:],
                                             rbcA[:])
                        nc.vector.tensor_mul(zT[p][64:128, gsl], cpB[:],
                                             rbcB[:])
                    else:
                        tmp = small.tile([128, 512], BF16, tag="tmp",
                                         name="tmp")
                        nc.vector.tensor_mul(tmp[0:64, :], cpA[:],
                                             rbcA[:])
                        nc.vector.tensor_mul(tmp[64:128, :], cpB[:],
                                             rbcB[:])
                        if g >= 1 and "ptmp" in pst:
                            cc = slice(g * 512, g * 512 + 1)
                            nc.vector.tensor_add(zT[p][:, cc], zT[p][:, cc],
                                                 pst["ptmp"][:, 511:512])
                        dst = zT[p][:, g * 512 + 1:g * 512 + 512]
                        nc.vector.tensor_add(dst, dst, tmp[:, 0:511])
                        pst["ptmp"] = tmp
                return chain

            def pass_chunks(ps_pool, py_pool, p, keysT, stats, qoff, pass_no):
                """Per-group unit chunks (software-pipelined): chunk[g] may
                only run once keysT/qpair cols < 512*(g+1) are final."""
                sts = {g: {} for g in range(NG)}
                pst = {}
                chunks = []
                pend_av = None
                pend_chain = None
                for g in range(NG):
                    cu = []
                    for j in range(4 * g + 4):
                        cu.append(make_unit(ps_pool, sts, p, keysT, qoff, g,
                                            j, pend_av))
                        pend_av = make_av(py_pool, sts, p, stats, g, j)
                        if j == 0 and pend_chain is not None:
                            cu.append(pend_chain)
                            pend_chain = None
                    pend_chain = make_chain(sts, pst, p, g, pass_no)
                    chunks.append(cu)
                chunks.append([pend_av, pend_chain])
                return chunks

            # e-prep subunits for pass 2 of pair p, key-group gp
            def eprep_units(aux_pool, p, gp):
                box = {}

                def sub_u():
                    et = etmp.tile([128, 512], BF16, tag="et", name="et")
                    nc.vector.tensor_sub(
                        et[:],
                        xsb[p][:, gp * 512 + 1:(gp + 1) * 512 + 1],
                        zT[p][:, gp * 512:(gp + 1) * 512])
                    box["et"] = et

                def tp_u(jj):
                    def th():
                        j = 4 * gp + jj
                        tp = aux_pool.tile([128, 512], BF16, tag="aux",
                                           name="tp")
                        nc.tensor.transpose(
                            tp[:, 0:128],
                            box["et"][:, jj * 128:(jj + 1) * 128],
                            ident_sb[:])
                        for hh in range(2):
                            nc.vector.tensor_copy(
                                eaug_sb[2 * p + hh][:, j * JW + 64:
                                                    j * JW + 128],
                                tp[:, hh * 64:hh * 64 + 64])
                    return th

                return [sub_u] + [tp_u(jj) for jj in range(4)]

            # k2 projection subunits for pair pp, t-slice n (reads resident x)
            def k2_units(aux_pool, pp, n):
                box = {}

                def mm_u(ci):
                    def th():
                        if ci == 0:
                            box["acc"] = aux_pool.tile([128, 512], F32,
                                                       tag="aux", name="k2acc")
                        for c in (2 * ci, 2 * ci + 1):
                            nc.tensor.matmul(
                                box["acc"][:],
                                wk2_sb[:, c * DG + pp * 128:
                                       c * DG + pp * 128 + 128],
                                xsb[c][:, n * 512:(n + 1) * 512],
                                start=(c == 0), stop=(c == 7),
                                skip_group_check=True)
                    return th

                def bias_u():
                    nc.vector.tensor_scalar_add(
                        k2pair[pp][:, n * 512:(n + 1) * 512],
                        box["acc"][:], bias_sb["bk2"][:, pp:pp + 1])

                return [mm_u(ci) for ci in range(4)] + [bias_u]

            def run_chunks(chunks, extras):
                sec = []
                for g in range(NG + 1):
                    sec += list(extras.get(g, []))
                    for u in chunks[g]:
                        u()
                        if sec:
                            sec.pop(0)()
                while sec:
                    sec.pop(0)()

            # ============ phase A: q,k projections + woven P1p0 ============
            from collections import deque
            wv = deque()

            def weave():
                if wv:
                    wv.popleft()()

            with tc.tile_pool(name="pproj", bufs=1, space="PSUM") as pproj, \
                 tc.tile_pool(name="psA", bufs=2, space="PSUM") as psA, \
                 tc.tile_pool(name="pyA", bufs=3, space="PSUM") as pyA:
                p0chunks = pass_chunks(psA, pyA, 0, kpair[0],
                                       (vaug_sb[0], vaug_sb[1]), 0, 1)
                plan = [(wq_sb, qpair, "bq"), (wk_sb, kpair, "bk")]

                def proj_slice(pp, n, w_sb, dsts, bnm):
                    def u():
                        acc = pproj.tile([128, 512], F32, tag="acc",
                                         name="acc")
                        for c in range(8):
                            nc.tensor.matmul(
                                acc[:],
                                w_sb[:, c * DG + pp * 128:
                                     c * DG + pp * 128 + 128],
                                xsb[c][:, n * 512:(n + 1) * 512],
                                start=(c == 0), stop=(c == 7),
                                skip_group_check=True)
                        nc.vector.tensor_scalar_add(
                            dsts[pp][:, n * 512:(n + 1) * 512],
                            acc[:], bias_sb[bnm][:, pp:pp + 1])
                    return u

                for n in range(NG):
                    # pair-0 q,k inline with weave pops
                    for w_sb, dsts, bnm in plan:
                        acc = pproj.tile([128, 512], F32, tag="acc",
                                         name="acc")
                        for c in range(8):
                            nc.tensor.matmul(
                                acc[:],
                                w_sb[:, c * DG:c * DG + 128],
                                xsb[c][:, n * 512:(n + 1) * 512],
                                start=(c == 0), stop=(c == 7),
                                skip_group_check=True)
                            weave()
                        nc.vector.tensor_scalar_add(
                            dsts[0][:, n * 512:(n + 1) * 512],
                            acc[:], bias_sb[bnm][:, 0:1])
                        weave()
                    wv.extend(p0chunks[n])
                    for w_sb, dsts, bnm in plan:
                        wv.append(proj_slice(1, n, w_sb, dsts, bnm))
                while wv:
                    wv.popleft()()
                for u in p0chunks[NG]:
                    u()

            # pad col for shifted q (col T = col T-1; value discarded)
            for p in range(NP):
                nc.vector.tensor_copy(qpair[p][:, T:T + 1],
                                      qpair[p][:, T - 1:T])

            # ============ phase B ============
            actx = contextlib.ExitStack()
            ps_pool = actx.enter_context(
                tc.tile_pool(name="ps", bufs=2, space="PSUM"))
            py_pool = actx.enter_context(
                tc.tile_pool(name="py", bufs=3, space="PSUM"))
            aux_pool = actx.enter_context(
                tc.tile_pool(name="aux", bufs=1, space="PSUM"))

            # P1p1: weave k2(p0) + eprep(p0, 0) late
            extras = {g: k2_units(aux_pool, 0, g) for g in range(NG)}
            extras[2] = extras[2] + eprep_units(aux_pool, 0, 0)
            run_chunks(pass_chunks(ps_pool, py_pool, 1, kpair[1],
                                   (vaug_sb[2], vaug_sb[3]), 0, 1), extras)

            # P2p0: weave k2(p1), eprep(p0, g) g=1..3, eprep(p1, 0) late
            extras = {g: k2_units(aux_pool, 1, g) for g in range(NG)}
            for gp in range(1, NG):
                extras[gp - 1] = extras[gp - 1] + eprep_units(aux_pool, 0, gp)
            extras[3] = extras[3] + eprep_units(aux_pool, 1, 0)
            run_chunks(pass_chunks(ps_pool, py_pool, 0, k2pair[0],
                                   (eaug_sb[0], eaug_sb[1]), 1, 2), extras)

            # P2p1: weave eprep(p1, g) g=1..3
            extras = {}
            for gp in range(1, NG):
                extras[gp - 1] = eprep_units(aux_pool, 1, gp)
            run_chunks(pass_chunks(ps_pool, py_pool, 1, k2pair[1],
                                   (eaug_sb[2], eaug_sb[3]), 1, 2), extras)

            # ============ tail: out projection ============
            actx.close()
            with tc.tile_pool(name="po", bufs=3, space="PSUM") as po, \
                 tc.tile_pool(name="ost", bufs=4) as ost:
                for cb in range(8):
                    accs = [po.tile([128, 1024], F32, tag="po", name="po")
                            for _ in range(2)]
                    for cc in range(NP):
                        for half in range(2):
                            for nn in range(2):
                                sl = slice(half * 1024 + nn * 512,
                                           half * 1024 + (nn + 1) * 512)
                                nc.tensor.matmul(
                                    accs[half][:, nn * 512:(nn + 1) * 512],
                                    wp_sb[:, cc * C + cb * 128:
                                          cc * C + cb * 128 + 128],
                                    zT[cc][:, sl],
                                    start=(cc == 0), stop=(cc == 1))
                    for half in range(2):
                        stg = ost.tile([128, 1024], BF16, tag="stg",
                                       name="stg")
                        if half == 0:
                            nc.vector.tensor_copy(stg[:], accs[half][:])
                            nc.sync.dma_start(
                                outT[cb * 128:(cb + 1) * 128, 0:1024], stg[:])
                        else:
                            nc.scalar.copy(stg[:], accs[half][:])
                            nc.gpsimd.dma_start(
                                outT[cb * 128:(cb + 1) * 128, 1024:2048],
                                stg[:])

    nc.compile()
    return nc


def _get_built():
    global _BUILT
    if _BUILT is None:
        _BUILT = _build()
    return _BUILT


def _prep_core(x, W_attn, b_attn, W_k2, b_k2, W_proj, core):
    bf16 = ml_dtypes.bfloat16
    b, hg = core // 4, core % 4
    cs = hg * DG
    xb = np.asarray(x[b], dtype=np.float32)
    # roll channels so this core's pair rows land at xT[0:256]
    xTr = np.ascontiguousarray(np.roll(xb.T, -cs, axis=0)).astype(bf16)
    xh = xb[:, cs:cs + DG]

    # vaugb: per head, [128 ki, NT tiles * JW]: col 0 = ones, 64:128 = v
    va = np.zeros((HPC, 128, NT, JW), np.float32)
    for h in range(HPC):
        va[h, :, :, 0] = 1.0
        va[h, :, :, 64:128] = xh[:, h * D:(h + 1) * D].reshape(NT, 128, D) \
            .transpose(1, 0, 2)
    vaug = np.ascontiguousarray(va.reshape(HPC * 128, NT * JW)).astype(bf16)

    def wslice(Wfull, c0):
        Wr = np.roll(Wfull, -cs, axis=0)   # match the x-row roll
        return np.ascontiguousarray(
            Wr[:, c0:c0 + DG].reshape(8, 128, DG).transpose(1, 0, 2)
            .reshape(128, 8 * DG)).astype(bf16)

    wq = wslice(W_attn, cs)
    wk = wslice(W_attn, C + cs)
    wk2 = wslice(W_k2, cs)
    wp = np.ascontiguousarray(
        W_proj[cs:cs + DG, :].reshape(2, 128, C).transpose(1, 0, 2)
        .reshape(128, 2 * C)).astype(bf16)

    bqv = np.ascontiguousarray(b_attn[cs:cs + DG].reshape(NP, 128).T)
    bkv = np.ascontiguousarray(b_attn[C + cs:C + cs + DG].reshape(NP, 128).T)
    bk2v = np.ascontiguousarray(b_k2[cs:cs + DG].reshape(NP, 128).T)

    return dict(xT=xTr, vaugb=vaug, wq=wq, wk=wk, wk2=wk2, wp=wp,
                bq=bqv.astype(np.float32), bk=bkv.astype(np.float32),
                bk2=bk2v.astype(np.float32))


def _consts():
    bf16 = ml_dtypes.bfloat16
    mask = np.ones((128, 128), np.float32)
    for kj in range(1, 128):
        mask[kj, :kj] = 0.0
    mask2 = np.concatenate([mask, mask], axis=1).astype(bf16)
    identb = np.eye(128, dtype=np.float32).astype(bf16)
    return dict(mask2=mask2, identb=identb)


def kernel(x, W_attn, b_attn, W_k2, b_k2, W_proj, b_proj):
    x = np.asarray(x, np.float32)
    W_attn = np.asarray(W_attn, np.float32)
    b_attn = np.asarray(b_attn, np.float32)
    W_k2 = np.asarray(W_k2, np.float32)
    b_k2 = np.asarray(b_k2, np.float32)
    W_proj = np.asarray(W_proj, np.float32)
    b_proj = np.asarray(b_proj, np.float32)

    cst = _consts()
    in_maps = []
    for core in range(8):
        m = _prep_core(x, W_attn, b_attn, W_k2, b_k2, W_proj, core)
        m.update(cst)
        in_maps.append(m)

    nc = _get_built()
    res = run_bass_kernel_spmd(nc, in_maps, list(range(8)))

    out = np.zeros((B, T, C), np.float32)
    for core in range(8):
        out[core // 4] += res.results[core]["outT"].astype(np.float32).T
    out += 2.0 * b_proj
    return out
